# revision 16
# baseline (speedup 1.0000x reference)
"""Trainium2 Bass kernel for ColorEntropyLoss (v8).

Math (per batch b, attention map s):
    cd[b,s,c] = sum_h attn[b,s,h] * (grid[b,h] == c)     # 10-bin weighted histogram
    p = cd / (sum_c cd + 1e-8);  entropy = -sum_c p*ln(p+1e-8);  out = mean

Sharding: pure data parallelism over batch B=512 across 8 NeuronCores
(64 batches/core, 8 groups of 8 batches; a group packs 128 SBUF
partitions as 8 batches x 16 maps). Entropy is computed host-side from
the DMA'd histograms (the cheap "all-reduce" of the sharding hint).

v8 design (trace-driven, over the v5 baseline):
- grid halves are the FIRST DMA on each HWDGE ring, so the full grid
  lands ~9.4us and the one-hot build never gates the pipeline.
- one-hot masks are COLOR-MAJOR per group-pair: ten is_equal
  tensor_scalar ops per pair, each a plain contiguous 512-col slice ->
  DVE 4x perf mode (~0.2us/op vs 1.8us/group for the tensor_tensor
  form). The matmul reads its 80 moving columns (c,b') through a
  strided 3-level AP into the c-major mask, so the PSUM column order
  (and the host decode) is unchanged.
- attn groups are split column-wise across BOTH HWDGE rings (sync has
  pixel chunks 0-15, scalar 16-31), so groups complete one at a time
  at aggregate bandwidth and the PE tracks the stream with ~0.2us lag.
  Group 7 lands as three pieces (12/16/4 chunks); the matmuls consume
  them in completion order, so only 4 matmuls trail the last HBM byte.
- the PE is pre-warmed with 32 dummy matmuls on a memset scratch tile
  so the HAM clock gate reaches 8/8 (2.4 GHz) right as the first real
  group lands.
- cd ships back as fp8e4 scaled by 1/16 on the ACT copy; the host
  entropy is scale-invariant (measured ~1e-4 rel err on the mean).
"""

import numpy as np
from contextlib import ExitStack

NUM_COLORS = 10
EPS = 1e-8
B, S, H, W = 512, 16, 64, 64
HW = H * W                      # 4096
N_CORES = 8
B_PER_CORE = B // N_CORES       # 64
N_GROUPS = 8                    # groups per core
B_PER_GROUP = B_PER_CORE // N_GROUPS  # 8 batches -> 128 partitions
P = 128
CHUNK = 128
N_CHUNKS = HW // CHUNK          # 32
NC80 = B_PER_GROUP * NUM_COLORS  # 80
GRID_COLS = N_GROUPS * N_CHUNKS * B_PER_GROUP  # 2048, col = g*256 + k*8 + b
PAIR_COLS = 2 * N_CHUNKS * B_PER_GROUP         # 512 grid cols per group-pair

USE_FP8 = True        # attn over the wire as fp8e4
OUT_FP8 = True        # cd back as fp8e4 (scaled 1/16 on the ACT copy)
N_WARM = 56           # dummy matmuls bridge until the first real group
                      # (~6us) so the HAM clock gate never re-throttles

_CACHE = {}


def _build_nc():
    import concourse.bacc as bacc
    import concourse.tile as tile
    import concourse.bass as bass
    from concourse import mybir

    f32 = mybir.dt.float32
    bf16 = mybir.dt.bfloat16
    fp8 = mybir.dt.float8e4
    attn_dt = fp8 if USE_FP8 else bf16
    out_dt = fp8 if OUT_FP8 else bf16
    OP = mybir.AluOpType
    AF = mybir.ActivationFunctionType

    nc = bacc.Bacc(
        "TRN2",
        target_bir_lowering=False,
        debug=False,
        num_devices=N_CORES,
        enable_partition_id=False,
    )

    attn_in = nc.dram_tensor(
        "attn_in", [P, N_GROUPS * HW], attn_dt, kind="ExternalInput"
    ).ap()
    grid_in = nc.dram_tensor(
        "grid_in", [P, GRID_COLS], bf16, kind="ExternalInput"
    ).ap()
    # raw 80-col histograms per group; host extracts the block diagonal
    cd_out = nc.dram_tensor(
        "cd_out", [P, N_GROUPS * NC80], out_dt, kind="ExternalOutput"
    ).ap()

    with tile.TileContext(nc) as tc:
        with ExitStack() as ctx:
            singles = ctx.enter_context(tc.tile_pool(name="singles", bufs=1))
            psum = ctx.enter_context(
                tc.tile_pool(name="psum", bufs=4, space="PSUM")
            )
            psum_d = ctx.enter_context(
                tc.tile_pool(name="psum_d", bufs=1, space="PSUM")
            )

            # ---- PE pre-warm: memset a scratch tile, then dummy matmuls
            # keep the PE busy from ~7.6us so the HAM clock gate is at 8/8
            # (2.4 GHz) when the first real group lands.
            dummy_w = singles.tile([P, P], attn_dt, name="dummy_w")
            nc.gpsimd.memset(dummy_w, 0)
            ps_d = psum_d.tile([P, 512], f32, name="ps_d")
            for _ in range(N_WARM):
                nc.tensor.matmul(ps_d[:, 0:P], dummy_w, dummy_w,
                                 start=True, stop=True)

            # ---- grid: first DMA on each HWDGE ring -> lands ~9.4us
            gridT = singles.tile([P, GRID_COLS], bf16)
            half = GRID_COLS // 2
            nc.sync.dma_start(out=gridT[:, 0:half], in_=grid_in[:, 0:half])
            nc.scalar.dma_start(
                out=gridT[:, half:GRID_COLS], in_=grid_in[:, half:GRID_COLS]
            )

            # ---- attn: one partition-major tile, few LARGE DMAs. With
            # more DMAs than the 8 completion-sem lanes, recycled issues
            # stall on full receipts and the stream's back half collapses
            # (measured: groups every ~1.8us instead of ~1.35us). Eleven
            # total DMAs keeps every lane's reuse ~8us after its receipt.
            attn_all = singles.tile([P, N_GROUPS * HW], attn_dt)

            def attn_dma(eng, c0, c1):
                eng.dma_start(out=attn_all[:, c0:c1], in_=attn_in[:, c0:c1])

            attn_dma(nc.sync, 0, 2 * HW)            # groups 0-1 (1MB)
            attn_dma(nc.scalar, 2 * HW, 4 * HW)     # groups 2-3 (1MB)
            attn_dma(nc.sync, 4 * HW, 6 * HW)       # groups 4-5 (1MB)
            attn_dma(nc.scalar, 6 * HW, 7 * HW)     # group 6 (512KB)
            g7 = 7 * HW
            # group 7 in three pieces; scalar's lands first, the final 4
            # chunks are a tiny 64KB DMA so almost no compute trails the
            # stream's last receipt
            attn_dma(nc.scalar, g7 + 16 * CHUNK, g7 + 28 * CHUNK)
            attn_dma(nc.sync, g7, g7 + 16 * CHUNK)
            attn_dma(nc.sync, g7 + 28 * CHUNK, g7 + 32 * CHUNK)

            cdq_all = singles.tile([P, N_GROUPS * NC80], out_dt)

            # ---- one-hot masks, color-major per group-pair ----
            # mask_pair[p, c*512 + g2*256 + k*8 + b] = (grid[p,(g,k,b)] == c)
            # Ten contiguous-slice is_equal tensor_scalar ops per pair:
            # single-src + bf16 + contiguous -> DVE 4x perf mode.
            mask_pairs = []
            for pr in range(N_GROUPS // 2):
                mq = singles.tile(
                    [P, NUM_COLORS * PAIR_COLS], bf16, name=f"maskp{pr}"
                )
                src = gridT[:, pr * PAIR_COLS : (pr + 1) * PAIR_COLS]
                for c in range(NUM_COLORS):
                    nc.vector.tensor_scalar(
                        out=mq[:, c * PAIR_COLS : (c + 1) * PAIR_COLS],
                        in0=src,
                        scalar1=float(c),
                        scalar2=None,
                        op0=OP.is_equal,
                    )
                mask_pairs.append(mq)

            def moving_ap(g, k):
                # 80 cols (c,b') for chunk k of group g: strided read into
                # the c-major pair mask; col order c*8+b matches v5 layout
                mq = mask_pairs[g // 2][:, :]
                return bass.AP(
                    tensor=mq.tensor,
                    offset=mq.offset + (g % 2) * (PAIR_COLS // 2)
                    + k * B_PER_GROUP,
                    ap=[mq.ap[0], [PAIR_COLS, NUM_COLORS], [1, B_PER_GROUP]],
                )

            for g in range(N_GROUPS):
                ps = psum.tile([P, 512], f32, name="ps", tag="ps")
                ps_c = ps[:, 0:NC80]
                if g < N_GROUPS - 1:
                    chunk_order = list(range(N_CHUNKS))
                else:
                    # match the 3-piece DMA completion order: scalar's
                    # 16-27 land first, then sync's 0-15, then 28-31
                    chunk_order = (
                        list(range(16, 28))
                        + list(range(0, 16))
                        + list(range(28, 32))
                    )
                for i, k in enumerate(chunk_order):
                    c0 = g * HW + k * CHUNK
                    nc.tensor.matmul(
                        ps_c,
                        attn_all[:, c0 : c0 + CHUNK],
                        moving_ap(g, k),
                        start=(i == 0),
                        stop=(i == N_CHUNKS - 1),
                    )

                # the only post-op: PSUM -> SBUF (fp8) on the ACT engine.
                # scale 1/16 keeps cd well inside fp8e4's range; the host
                # entropy is scale-invariant so no decode step is needed.
                nc.scalar.activation(
                    cdq_all[:, g * NC80 : (g + 1) * NC80], ps_c, AF.Copy,
                    scale=(1.0 / 16.0 if OUT_FP8 else 1.0),
                )
                if g == N_GROUPS - 2:
                    # ship groups 0-6 while group 7 computes; only the last
                    # 80 columns remain on the critical tail
                    cut = (N_GROUPS - 1) * NC80
                    nc.sync.dma_start(
                        out=cd_out[:, 0:cut], in_=cdq_all[:, 0:cut]
                    )

            # last piece goes out on the ACT ring right behind its copy
            cut = (N_GROUPS - 1) * NC80
            nc.scalar.dma_start(
                out=cd_out[:, cut : N_GROUPS * NC80],
                in_=cdq_all[:, cut : N_GROUPS * NC80],
            )

    nc.compile()
    return nc


def _get_nc():
    if "nc" not in _CACHE:
        _CACHE["nc"] = _build_nc()
    return _CACHE["nc"]


def _make_in_maps(attn_weights, grids):
    import ml_dtypes

    attn_np_dt = ml_dtypes.float8_e4m3 if USE_FP8 else ml_dtypes.bfloat16
    attn = np.asarray(attn_weights, dtype=np.float32)   # [512,16,64,64]
    grid = np.asarray(grids)                            # [512,64,64]
    in_maps = []
    for c in range(N_CORES):
        lo = c * B_PER_CORE
        # (g,b,s,k,p) -> (p,g,k,b,s): row p, col g*4096 + k*128 + b*16 + s
        a = attn[lo : lo + B_PER_CORE].reshape(
            N_GROUPS, B_PER_GROUP, S, N_CHUNKS, CHUNK
        )
        a = np.ascontiguousarray(a.transpose(4, 0, 3, 1, 2)).reshape(
            CHUNK, N_GROUPS * HW
        )
        # (g,b,k,p) -> (p,g,k,b): col g*256 + k*8 + b
        g_ = grid[lo : lo + B_PER_CORE].reshape(
            N_GROUPS, B_PER_GROUP, N_CHUNKS, CHUNK
        )
        g_ = np.ascontiguousarray(g_.transpose(3, 0, 2, 1)).reshape(
            CHUNK, GRID_COLS
        )
        in_maps.append(
            {
                "attn_in": a.astype(attn_np_dt),
                "grid_in": g_.astype(np.float32).astype(ml_dtypes.bfloat16),
            }
        )
    return in_maps


def kernel(attn_weights: np.ndarray, grids: np.ndarray) -> np.ndarray:
    from concourse.bass_utils import run_bass_kernel_spmd

    nc = _get_nc()
    in_maps = _make_in_maps(attn_weights, grids)
    res = run_bass_kernel_spmd(nc, in_maps, core_ids=list(range(N_CORES)))

    # Host finale: pick the block diagonal (row p=(b,s) owns batch p//16
    # of its group), then entropy + mean in f64.
    b_idx = (np.arange(P) // S)[:, None, None, None]     # [128,1,1,1]
    total = 0.0
    for c in range(N_CORES):
        r = res.results[c]["cd_out"].astype(np.float64)  # [128, 640]
        r = r.reshape(P, N_GROUPS, NUM_COLORS, B_PER_GROUP)
        cd = np.take_along_axis(r, b_idx, axis=3)[..., 0]  # [128, 8, 10]
        s_ = cd.sum(-1) + EPS
        p_ = cd / s_[..., None]
        ent = -(p_ * np.log(p_ + EPS)).sum(-1)
        total += float(ent.sum())
    return np.float32(total / (B * S))


# revision 20
# speedup vs baseline: 1.1497x; 1.1497x over previous
"""Trainium2 Bass kernel for ColorEntropyLoss (v8).

Math (per batch b, attention map s):
    cd[b,s,c] = sum_h attn[b,s,h] * (grid[b,h] == c)     # 10-bin weighted histogram
    p = cd / (sum_c cd + 1e-8);  entropy = -sum_c p*ln(p+1e-8);  out = mean

Sharding: pure data parallelism over batch B=512 across 8 NeuronCores
(64 batches/core, 8 groups of 8 batches; a group packs 128 SBUF
partitions as 8 batches x 16 maps). Entropy is computed host-side from
the DMA'd histograms (the cheap "all-reduce" of the sharding hint).

v8 design (trace-driven, over the v5 baseline):
- grid halves are the FIRST DMA on each HWDGE ring, so the full grid
  lands ~9.4us and the one-hot build never gates the pipeline.
- one-hot masks are COLOR-MAJOR per group-pair: ten is_equal
  tensor_scalar ops per pair, each a plain contiguous 512-col slice ->
  DVE 4x perf mode (~0.2us/op vs 1.8us/group for the tensor_tensor
  form). The matmul reads its 80 moving columns (c,b') through a
  strided 3-level AP into the c-major mask, so the PSUM column order
  (and the host decode) is unchanged.
- attn groups are split column-wise across BOTH HWDGE rings (sync has
  pixel chunks 0-15, scalar 16-31), so groups complete one at a time
  at aggregate bandwidth and the PE tracks the stream with ~0.2us lag.
  Group 7 lands as three pieces (12/16/4 chunks); the matmuls consume
  them in completion order, so only 4 matmuls trail the last HBM byte.
- the PE is pre-warmed with 32 dummy matmuls on a memset scratch tile
  so the HAM clock gate reaches 8/8 (2.4 GHz) right as the first real
  group lands.
- cd ships back as fp8e4 scaled by 1/16 on the ACT copy; the host
  entropy is scale-invariant (measured ~1e-4 rel err on the mean).
"""

import numpy as np
from contextlib import ExitStack

NUM_COLORS = 10
EPS = 1e-8
B, S, H, W = 512, 16, 64, 64
HW = H * W                      # 4096
N_CORES = 8
B_PER_CORE = B // N_CORES       # 64
N_GROUPS = 8                    # groups per core
B_PER_GROUP = B_PER_CORE // N_GROUPS  # 8 batches -> 128 partitions
P = 128
CHUNK = 128
N_CHUNKS = HW // CHUNK          # 32
NC80 = B_PER_GROUP * NUM_COLORS  # 80
GRID_COLS = N_GROUPS * N_CHUNKS * B_PER_GROUP  # 2048, col = g*256 + k*8 + b
PAIR_COLS = 2 * N_CHUNKS * B_PER_GROUP         # 512 grid cols per group-pair

USE_FP8 = True        # attn over the wire as fp8e4
OUT_FP8 = True        # cd back as fp8e4 (scaled 1/16 on the ACT copy)
N_WARM = 44           # dummy matmuls bridge until the first real group
                      # so the HAM clock gate never re-throttles

_CACHE = {}


def _build_nc():
    import concourse.bacc as bacc
    import concourse.tile as tile
    import concourse.bass as bass
    from concourse import mybir

    f32 = mybir.dt.float32
    bf16 = mybir.dt.bfloat16
    fp8 = mybir.dt.float8e4
    attn_dt = fp8 if USE_FP8 else bf16
    out_dt = fp8 if OUT_FP8 else bf16
    OP = mybir.AluOpType
    AF = mybir.ActivationFunctionType

    nc = bacc.Bacc(
        "TRN2",
        target_bir_lowering=False,
        debug=False,
        num_devices=N_CORES,
        enable_partition_id=False,
    )

    attn_in = nc.dram_tensor(
        "attn_in", [N_GROUPS * P, HW], attn_dt, kind="ExternalInput"
    ).ap()
    grid_in = nc.dram_tensor(
        "grid_in", [P, GRID_COLS], bf16, kind="ExternalInput"
    ).ap()
    # raw 80-col histograms per group; host extracts the block diagonal
    cd_out = nc.dram_tensor(
        "cd_out", [P, N_GROUPS * NC80], out_dt, kind="ExternalOutput"
    ).ap()

    with tile.TileContext(nc) as tc:
        with ExitStack() as ctx:
            singles = ctx.enter_context(tc.tile_pool(name="singles", bufs=1))
            psum = ctx.enter_context(
                tc.tile_pool(name="psum", bufs=4, space="PSUM")
            )
            psum_d = ctx.enter_context(
                tc.tile_pool(name="psum_d", bufs=1, space="PSUM")
            )

            # ---- PE pre-warm: memset a scratch tile, then dummy matmuls
            # keep the PE busy from ~7.6us so the HAM clock gate is at 8/8
            # (2.4 GHz) when the first real group lands.
            dummy_w = singles.tile([P, P], attn_dt, name="dummy_w")
            nc.gpsimd.memset(dummy_w, 0)
            ps_d = psum_d.tile([P, 512], f32, name="ps_d")
            for _ in range(N_WARM):
                nc.tensor.matmul(ps_d[:, 0:P], dummy_w, dummy_w,
                                 start=True, stop=True)

            # ---- grid: four 128KB pieces, first on each HWDGE ring; the
            # pair-p mask ops wait only on piece p, and small pieces get
            # their completion receipts back ~1us sooner than halves
            gridT = singles.tile([P, GRID_COLS], bf16)
            Q = GRID_COLS // 4
            nc.sync.dma_start(out=gridT[:, 0:Q], in_=grid_in[:, 0:Q])
            nc.scalar.dma_start(
                out=gridT[:, Q : 2 * Q], in_=grid_in[:, Q : 2 * Q]
            )
            nc.sync.dma_start(
                out=gridT[:, 2 * Q : 3 * Q], in_=grid_in[:, 2 * Q : 3 * Q]
            )
            nc.scalar.dma_start(
                out=gridT[:, 3 * Q : GRID_COLS], in_=grid_in[:, 3 * Q :]
            )

            # ---- attn: ONE whole-group 512KB DMA per group, alternating
            # rings. 13 DMAs total keeps completion-sem-lane recycling off
            # the stream (v8's 21 DMAs measured periodic issue stalls that
            # collapsed the back half to ~1.8us/group).
            attn_sb = []
            for g in range(N_GROUPS):
                t = singles.tile([P, HW], attn_dt, name=f"attn{g}")
                attn_sb.append(t)
                rows = slice(g * P, (g + 1) * P)
                if g < N_GROUPS - 1:
                    ring = nc.sync if g % 2 == 0 else nc.scalar
                    ring.dma_start(out=t, in_=attn_in[rows, :])
                else:
                    # last group in three pieces; the final 4 chunks are a
                    # tiny 64KB DMA so almost no compute trails the stream
                    cut = 16 * CHUNK
                    cut2 = 28 * CHUNK
                    nc.scalar.dma_start(
                        out=t[:, cut:cut2], in_=attn_in[rows, cut:cut2]
                    )
                    nc.sync.dma_start(
                        out=t[:, 0:cut], in_=attn_in[rows, 0:cut]
                    )
                    nc.sync.dma_start(
                        out=t[:, cut2:HW], in_=attn_in[rows, cut2:HW]
                    )

            cdq_all = singles.tile([P, N_GROUPS * NC80], out_dt)

            # ---- one-hot masks, color-major per group-pair ----
            # mask_pair[p, c*512 + g2*256 + k*8 + b] = (grid[p,(g,k,b)] == c)
            # Ten contiguous-slice is_equal tensor_scalar ops per pair:
            # single-src + bf16 + contiguous -> DVE 4x perf mode.
            mask_pairs = []
            for pr in range(N_GROUPS // 2):
                mq = singles.tile(
                    [P, NUM_COLORS * PAIR_COLS], bf16, name=f"maskp{pr}"
                )
                src = gridT[:, pr * PAIR_COLS : (pr + 1) * PAIR_COLS]
                for c in range(NUM_COLORS):
                    nc.vector.tensor_scalar(
                        out=mq[:, c * PAIR_COLS : (c + 1) * PAIR_COLS],
                        in0=src,
                        scalar1=float(c),
                        scalar2=None,
                        op0=OP.is_equal,
                    )
                mask_pairs.append(mq)

            def moving_ap(g, k):
                # 80 cols (c,b') for chunk k of group g: strided read into
                # the c-major pair mask; col order c*8+b matches v5 layout
                mq = mask_pairs[g // 2][:, :]
                return bass.AP(
                    tensor=mq.tensor,
                    offset=mq.offset + (g % 2) * (PAIR_COLS // 2)
                    + k * B_PER_GROUP,
                    ap=[mq.ap[0], [PAIR_COLS, NUM_COLORS], [1, B_PER_GROUP]],
                )

            for g in range(N_GROUPS):
                attn_bf = attn_sb[g]

                ps = psum.tile([P, 512], f32, name="ps", tag="ps")
                ps_c = ps[:, 0:NC80]
                if g < N_GROUPS - 1:
                    chunk_order = list(range(N_CHUNKS))
                else:
                    # match the 3-piece DMA completion order: scalar's
                    # 16-27 land first, then sync's 0-15, then 28-31
                    chunk_order = (
                        list(range(16, 28))
                        + list(range(0, 16))
                        + list(range(28, 32))
                    )
                for i, k in enumerate(chunk_order):
                    nc.tensor.matmul(
                        ps_c,
                        attn_bf[:, k * CHUNK : (k + 1) * CHUNK],
                        moving_ap(g, k),
                        start=(i == 0),
                        stop=(i == N_CHUNKS - 1),
                    )

                # the only post-op: PSUM -> SBUF (fp8) on the ACT engine.
                # scale 1/16 keeps cd well inside fp8e4's range; the host
                # entropy is scale-invariant so no decode step is needed.
                nc.scalar.activation(
                    cdq_all[:, g * NC80 : (g + 1) * NC80], ps_c, AF.Copy,
                    scale=(1.0 / 16.0 if OUT_FP8 else 1.0),
                )
                if g == N_GROUPS - 2:
                    # ship groups 0-6 while group 7 computes; only the last
                    # 80 columns remain on the critical tail
                    cut = (N_GROUPS - 1) * NC80
                    nc.sync.dma_start(
                        out=cd_out[:, 0:cut], in_=cdq_all[:, 0:cut]
                    )

            # last piece goes out on the ACT ring right behind its copy
            cut = (N_GROUPS - 1) * NC80
            nc.scalar.dma_start(
                out=cd_out[:, cut : N_GROUPS * NC80],
                in_=cdq_all[:, cut : N_GROUPS * NC80],
            )

    nc.compile()
    return nc


def _get_nc():
    if "nc" not in _CACHE:
        _CACHE["nc"] = _build_nc()
    return _CACHE["nc"]


def _make_in_maps(attn_weights, grids):
    import ml_dtypes

    attn_np_dt = ml_dtypes.float8_e4m3 if USE_FP8 else ml_dtypes.bfloat16
    attn = np.asarray(attn_weights, dtype=np.float32)   # [512,16,64,64]
    grid = np.asarray(grids)                            # [512,64,64]
    in_maps = []
    for c in range(N_CORES):
        lo = c * B_PER_CORE
        # (g,b,s,k,p) -> (g,p,k,b,s): row g*128+p, col k*128 + b*16 + s
        a = attn[lo : lo + B_PER_CORE].reshape(
            N_GROUPS, B_PER_GROUP, S, N_CHUNKS, CHUNK
        )
        a = np.ascontiguousarray(a.transpose(0, 4, 3, 1, 2)).reshape(
            N_GROUPS * CHUNK, HW
        )
        # (g,b,k,p) -> (p,g,k,b): col g*256 + k*8 + b
        g_ = grid[lo : lo + B_PER_CORE].reshape(
            N_GROUPS, B_PER_GROUP, N_CHUNKS, CHUNK
        )
        g_ = np.ascontiguousarray(g_.transpose(3, 0, 2, 1)).reshape(
            CHUNK, GRID_COLS
        )
        in_maps.append(
            {
                "attn_in": a.astype(attn_np_dt),
                "grid_in": g_.astype(np.float32).astype(ml_dtypes.bfloat16),
            }
        )
    return in_maps


def kernel(attn_weights: np.ndarray, grids: np.ndarray) -> np.ndarray:
    from concourse.bass_utils import run_bass_kernel_spmd

    nc = _get_nc()
    in_maps = _make_in_maps(attn_weights, grids)
    res = run_bass_kernel_spmd(nc, in_maps, core_ids=list(range(N_CORES)))

    # Host finale: pick the block diagonal (row p=(b,s) owns batch p//16
    # of its group), then entropy + mean in f64.
    b_idx = (np.arange(P) // S)[:, None, None, None]     # [128,1,1,1]
    total = 0.0
    for c in range(N_CORES):
        r = res.results[c]["cd_out"].astype(np.float64)  # [128, 640]
        r = r.reshape(P, N_GROUPS, NUM_COLORS, B_PER_GROUP)
        cd = np.take_along_axis(r, b_idx, axis=3)[..., 0]  # [128, 8, 10]
        s_ = cd.sum(-1) + EPS
        p_ = cd / s_[..., None]
        ent = -(p_ * np.log(p_ + EPS)).sum(-1)
        total += float(ent.sum())
    return np.float32(total / (B * S))


# revision 25
# speedup vs baseline: 1.1871x; 1.0326x over previous
"""Trainium2 Bass kernel for ColorEntropyLoss (v8).

Math (per batch b, attention map s):
    cd[b,s,c] = sum_h attn[b,s,h] * (grid[b,h] == c)     # 10-bin weighted histogram
    p = cd / (sum_c cd + 1e-8);  entropy = -sum_c p*ln(p+1e-8);  out = mean

Sharding: pure data parallelism over batch B=512 across 8 NeuronCores
(64 batches/core, 8 groups of 8 batches; a group packs 128 SBUF
partitions as 8 batches x 16 maps). Entropy is computed host-side from
the DMA'd histograms (the cheap "all-reduce" of the sharding hint).

v8 design (trace-driven, over the v5 baseline):
- grid halves are the FIRST DMA on each HWDGE ring, so the full grid
  lands ~9.4us and the one-hot build never gates the pipeline.
- one-hot masks are COLOR-MAJOR per group-pair: ten is_equal
  tensor_scalar ops per pair, each a plain contiguous 512-col slice ->
  DVE 4x perf mode (~0.2us/op vs 1.8us/group for the tensor_tensor
  form). The matmul reads its 80 moving columns (c,b') through a
  strided 3-level AP into the c-major mask, so the PSUM column order
  (and the host decode) is unchanged.
- attn groups are split column-wise across BOTH HWDGE rings (sync has
  pixel chunks 0-15, scalar 16-31), so groups complete one at a time
  at aggregate bandwidth and the PE tracks the stream with ~0.2us lag.
  Group 7 lands as three pieces (12/16/4 chunks); the matmuls consume
  them in completion order, so only 4 matmuls trail the last HBM byte.
- the PE is pre-warmed with 32 dummy matmuls on a memset scratch tile
  so the HAM clock gate reaches 8/8 (2.4 GHz) right as the first real
  group lands.
- cd ships back as fp8e4 scaled by 1/16 on the ACT copy; the host
  entropy is scale-invariant (measured ~1e-4 rel err on the mean).
"""

import numpy as np
from contextlib import ExitStack

NUM_COLORS = 10
EPS = 1e-8
B, S, H, W = 512, 16, 64, 64
HW = H * W                      # 4096
N_CORES = 8
B_PER_CORE = B // N_CORES       # 64
N_GROUPS = 8                    # groups per core
B_PER_GROUP = B_PER_CORE // N_GROUPS  # 8 batches -> 128 partitions
P = 128
CHUNK = 128
N_CHUNKS = HW // CHUNK          # 32
NC80 = B_PER_GROUP * NUM_COLORS  # 80
GRID_COLS = N_GROUPS * N_CHUNKS * B_PER_GROUP  # 2048, col = g*256 + k*8 + b
PAIR_COLS = 2 * N_CHUNKS * B_PER_GROUP         # 512 grid cols per group-pair

USE_FP8 = True        # attn over the wire as fp8e4
OUT_FP8 = True        # cd back as fp8e4 (scaled 1/16 on the ACT copy)
N_WARM = 56           # dummy matmuls bridge until the first real group;
                      # measured: even a ~1.2us PE idle gap can lose the
                      # HAM 2.4GHz state, so overshoot (56ns/dummy) is cheap

_CACHE = {}


def _build_nc():
    import concourse.bacc as bacc
    import concourse.tile as tile
    import concourse.bass as bass
    from concourse import mybir

    f32 = mybir.dt.float32
    bf16 = mybir.dt.bfloat16
    fp8 = mybir.dt.float8e4
    attn_dt = fp8 if USE_FP8 else bf16
    out_dt = fp8 if OUT_FP8 else bf16
    OP = mybir.AluOpType
    AF = mybir.ActivationFunctionType

    nc = bacc.Bacc(
        "TRN2",
        target_bir_lowering=False,
        debug=False,
        num_devices=N_CORES,
        enable_partition_id=False,
    )

    attn_in = nc.dram_tensor(
        "attn_in", [N_GROUPS * P, HW], attn_dt, kind="ExternalInput"
    ).ap()
    grid_in = nc.dram_tensor(
        "grid_in", [P, GRID_COLS], bf16, kind="ExternalInput"
    ).ap()
    # raw 80-col histograms per group; host extracts the block diagonal
    cd_out = nc.dram_tensor(
        "cd_out", [P, N_GROUPS * NC80], out_dt, kind="ExternalOutput"
    ).ap()

    with tile.TileContext(nc) as tc:
        with ExitStack() as ctx:
            singles = ctx.enter_context(tc.tile_pool(name="singles", bufs=1))
            psum = ctx.enter_context(
                tc.tile_pool(name="psum", bufs=4, space="PSUM")
            )
            psum_d = ctx.enter_context(
                tc.tile_pool(name="psum_d", bufs=1, space="PSUM")
            )

            # ---- PE pre-warm: memset a scratch tile, then dummy matmuls
            # keep the PE busy from ~7.6us so the HAM clock gate is at 8/8
            # (2.4 GHz) when the first real group lands.
            dummy_w = singles.tile([P, P], attn_dt, name="dummy_w")
            nc.gpsimd.memset(dummy_w, 0)
            ps_d = psum_d.tile([P, 512], f32, name="ps_d")
            for _ in range(N_WARM):
                nc.tensor.matmul(ps_d[:, 0:P], dummy_w, dummy_w,
                                 start=True, stop=True)

            # ---- grid: four 128KB quarters, first on each ring; the
            # pair-p masks depend only on quarter p (separate sems), so
            # the chain starts ~9.7us (measured) instead of ~11us
            gridT = singles.tile([P, GRID_COLS], bf16)
            Q = GRID_COLS // 4
            nc.sync.dma_start(out=gridT[:, 0:Q], in_=grid_in[:, 0:Q])
            nc.scalar.dma_start(
                out=gridT[:, Q : 2 * Q], in_=grid_in[:, Q : 2 * Q]
            )
            nc.sync.dma_start(
                out=gridT[:, 2 * Q : 3 * Q], in_=grid_in[:, 2 * Q : 3 * Q]
            )
            nc.scalar.dma_start(
                out=gridT[:, 3 * Q : GRID_COLS], in_=grid_in[:, 3 * Q :]
            )

            # ---- attn: whole-group 512KB DMAs (4KB contiguous rows ramp
            # the SDMA stream to ~400GB/s ~2us faster than 2KB column
            # splits), alternating rings. Tile deps are per-TILE, so the
            # last group uses three SEPARATE tiles: only the 4-chunk 64KB
            # tail piece's matmuls wait on the stream's final receipt.
            attn_sb = []
            for g in range(6):
                t = singles.tile([P, HW], attn_dt, name=f"attn{g}")
                attn_sb.append(t)
                rows = slice(g * P, (g + 1) * P)
                ring = nc.sync if g % 2 == 0 else nc.scalar
                ring.dma_start(out=t, in_=attn_in[rows, :])
            # groups 6 and 7 land last; finer pieces (own tiles) so the
            # PE's final matmuls gate on small, early-receipt transfers
            g6r = slice(6 * P, 7 * P)
            t6a = singles.tile([P, 16 * CHUNK], attn_dt, name="attn6a")
            t6b = singles.tile([P, 16 * CHUNK], attn_dt, name="attn6b")
            nc.sync.dma_start(out=t6a, in_=attn_in[g6r, 0 : 16 * CHUNK])
            nc.scalar.dma_start(out=t6b, in_=attn_in[g6r, 16 * CHUNK : HW])
            g7r = slice(7 * P, 8 * P)
            K1, K2 = 14, 28  # piece boundaries (in chunks)
            t7a = singles.tile([P, K1 * CHUNK], attn_dt, name="attn7a")
            t7b = singles.tile([P, (K2 - K1) * CHUNK], attn_dt, name="attn7b")
            t7c = singles.tile([P, (N_CHUNKS - K2) * CHUNK], attn_dt,
                               name="attn7c")
            nc.scalar.dma_start(out=t7a, in_=attn_in[g7r, 0 : K1 * CHUNK])
            nc.sync.dma_start(
                out=t7b, in_=attn_in[g7r, K1 * CHUNK : K2 * CHUNK]
            )
            nc.scalar.dma_start(out=t7c, in_=attn_in[g7r, K2 * CHUNK : HW])

            def stat_slice(g, k):
                # stationary operand for (group, chunk) over the piece tiles
                if g < 6:
                    return attn_sb[g][:, k * CHUNK : (k + 1) * CHUNK]
                if g == 6:
                    t, k0 = (t6a, 0) if k < 16 else (t6b, 16)
                else:
                    t, k0 = (
                        (t7a, 0) if k < K1
                        else (t7b, K1) if k < K2
                        else (t7c, K2)
                    )
                return t[:, (k - k0) * CHUNK : (k - k0 + 1) * CHUNK]

            cdq_all = singles.tile([P, N_GROUPS * NC80], out_dt)

            # ---- one-hot masks, color-major per group-pair ----
            # mask_pair[p, c*512 + g2*256 + k*8 + b] = (grid[p,(g,k,b)] == c)
            # Ten contiguous-slice is_equal tensor_scalar ops per pair:
            # single-src + bf16 + contiguous -> DVE 4x perf mode.
            mask_pairs = []
            for pr in range(N_GROUPS // 2):
                mq = singles.tile(
                    [P, NUM_COLORS * PAIR_COLS], bf16, name=f"maskp{pr}"
                )
                src = gridT[:, pr * PAIR_COLS : (pr + 1) * PAIR_COLS]
                for c in range(NUM_COLORS):
                    nc.vector.tensor_scalar(
                        out=mq[:, c * PAIR_COLS : (c + 1) * PAIR_COLS],
                        in0=src,
                        scalar1=float(c),
                        scalar2=None,
                        op0=OP.is_equal,
                    )
                mask_pairs.append(mq)

            def moving_ap(g, k):
                # 80 cols (c,b') for chunk k of group g: strided read into
                # the c-major pair mask; col order c*8+b matches v5 layout
                mq = mask_pairs[g // 2][:, :]
                return bass.AP(
                    tensor=mq.tensor,
                    offset=mq.offset + (g % 2) * (PAIR_COLS // 2)
                    + k * B_PER_GROUP,
                    ap=[mq.ap[0], [PAIR_COLS, NUM_COLORS], [1, B_PER_GROUP]],
                )

            for g in range(N_GROUPS):
                ps = psum.tile([P, 512], f32, name="ps", tag="ps")
                ps_c = ps[:, 0:NC80]
                for k in range(N_CHUNKS):
                    nc.tensor.matmul(
                        ps_c,
                        stat_slice(g, k),
                        moving_ap(g, k),
                        start=(k == 0),
                        stop=(k == N_CHUNKS - 1),
                    )

                # the only post-op: PSUM -> SBUF (fp8) on the ACT engine.
                # scale 1/16 keeps cd well inside fp8e4's range; the host
                # entropy is scale-invariant so no decode step is needed.
                nc.scalar.activation(
                    cdq_all[:, g * NC80 : (g + 1) * NC80], ps_c, AF.Copy,
                    scale=(1.0 / 16.0 if OUT_FP8 else 1.0),
                )
                if g == N_GROUPS - 2:
                    # ship groups 0-6 while group 7 computes; only the last
                    # 80 columns remain on the critical tail
                    cut = (N_GROUPS - 1) * NC80
                    nc.sync.dma_start(
                        out=cd_out[:, 0:cut], in_=cdq_all[:, 0:cut]
                    )

            # last piece goes out on the ACT ring right behind its copy
            cut = (N_GROUPS - 1) * NC80
            nc.scalar.dma_start(
                out=cd_out[:, cut : N_GROUPS * NC80],
                in_=cdq_all[:, cut : N_GROUPS * NC80],
            )

    nc.compile()
    return nc


def _get_nc():
    if "nc" not in _CACHE:
        _CACHE["nc"] = _build_nc()
    return _CACHE["nc"]


def _make_in_maps(attn_weights, grids):
    import ml_dtypes

    attn_np_dt = ml_dtypes.float8_e4m3 if USE_FP8 else ml_dtypes.bfloat16
    attn = np.asarray(attn_weights, dtype=np.float32)   # [512,16,64,64]
    grid = np.asarray(grids)                            # [512,64,64]
    in_maps = []
    for c in range(N_CORES):
        lo = c * B_PER_CORE
        # (g,b,s,k,p) -> (g,p,k,b,s): row g*128+p, col k*128 + b*16 + s
        a = attn[lo : lo + B_PER_CORE].reshape(
            N_GROUPS, B_PER_GROUP, S, N_CHUNKS, CHUNK
        )
        a = np.ascontiguousarray(a.transpose(0, 4, 3, 1, 2)).reshape(
            N_GROUPS * CHUNK, HW
        )
        # (g,b,k,p) -> (p,g,k,b): col g*256 + k*8 + b
        g_ = grid[lo : lo + B_PER_CORE].reshape(
            N_GROUPS, B_PER_GROUP, N_CHUNKS, CHUNK
        )
        g_ = np.ascontiguousarray(g_.transpose(3, 0, 2, 1)).reshape(
            CHUNK, GRID_COLS
        )
        in_maps.append(
            {
                "attn_in": a.astype(attn_np_dt),
                "grid_in": g_.astype(np.float32).astype(ml_dtypes.bfloat16),
            }
        )
    return in_maps


def kernel(attn_weights: np.ndarray, grids: np.ndarray) -> np.ndarray:
    from concourse.bass_utils import run_bass_kernel_spmd

    nc = _get_nc()
    in_maps = _make_in_maps(attn_weights, grids)
    res = run_bass_kernel_spmd(nc, in_maps, core_ids=list(range(N_CORES)))

    # Host finale: pick the block diagonal (row p=(b,s) owns batch p//16
    # of its group), then entropy + mean in f64.
    b_idx = (np.arange(P) // S)[:, None, None, None]     # [128,1,1,1]
    total = 0.0
    for c in range(N_CORES):
        r = res.results[c]["cd_out"].astype(np.float64)  # [128, 640]
        r = r.reshape(P, N_GROUPS, NUM_COLORS, B_PER_GROUP)
        cd = np.take_along_axis(r, b_idx, axis=3)[..., 0]  # [128, 8, 10]
        s_ = cd.sum(-1) + EPS
        p_ = cd / s_[..., None]
        ent = -(p_ * np.log(p_ + EPS)).sum(-1)
        total += float(ent.sum())
    return np.float32(total / (B * S))


# revision 29
# speedup vs baseline: 1.2518x; 1.0545x over previous
"""Trainium2 Bass kernel for ColorEntropyLoss (v8).

Math (per batch b, attention map s):
    cd[b,s,c] = sum_h attn[b,s,h] * (grid[b,h] == c)     # 10-bin weighted histogram
    p = cd / (sum_c cd + 1e-8);  entropy = -sum_c p*ln(p+1e-8);  out = mean

Sharding: pure data parallelism over batch B=512 across 8 NeuronCores
(64 batches/core, 8 groups of 8 batches; a group packs 128 SBUF
partitions as 8 batches x 16 maps). Entropy is computed host-side from
the DMA'd histograms (the cheap "all-reduce" of the sharding hint).

v8 design (trace-driven, over the v5 baseline):
- grid halves are the FIRST DMA on each HWDGE ring, so the full grid
  lands ~9.4us and the one-hot build never gates the pipeline.
- one-hot masks are COLOR-MAJOR per group-pair: ten is_equal
  tensor_scalar ops per pair, each a plain contiguous 512-col slice ->
  DVE 4x perf mode (~0.2us/op vs 1.8us/group for the tensor_tensor
  form). The matmul reads its 80 moving columns (c,b') through a
  strided 3-level AP into the c-major mask, so the PSUM column order
  (and the host decode) is unchanged.
- attn groups are split column-wise across BOTH HWDGE rings (sync has
  pixel chunks 0-15, scalar 16-31), so groups complete one at a time
  at aggregate bandwidth and the PE tracks the stream with ~0.2us lag.
  Group 7 lands as three pieces (12/16/4 chunks); the matmuls consume
  them in completion order, so only 4 matmuls trail the last HBM byte.
- the PE is pre-warmed with 32 dummy matmuls on a memset scratch tile
  so the HAM clock gate reaches 8/8 (2.4 GHz) right as the first real
  group lands.
- cd ships back as fp8e4 scaled by 1/16 on the ACT copy; the host
  entropy is scale-invariant (measured ~1e-4 rel err on the mean).
"""

import numpy as np
from contextlib import ExitStack

NUM_COLORS = 10
EPS = 1e-8
B, S, H, W = 512, 16, 64, 64
HW = H * W                      # 4096
N_CORES = 8
B_PER_CORE = B // N_CORES       # 64
N_GROUPS = 8                    # groups per core
B_PER_GROUP = B_PER_CORE // N_GROUPS  # 8 batches -> 128 partitions
P = 128
CHUNK = 128
N_CHUNKS = HW // CHUNK          # 32
NC80 = B_PER_GROUP * NUM_COLORS  # 80
GRID_COLS = N_GROUPS * N_CHUNKS * B_PER_GROUP  # 2048, col = g*256 + k*8 + b
PAIR_COLS = 2 * N_CHUNKS * B_PER_GROUP         # 512 grid cols per group-pair

USE_FP8 = True        # attn over the wire as fp8e4
OUT_FP8 = True        # cd back as fp8e4 (scaled 1/16 on the ACT copy)
N_WARM = 56           # dummy matmuls bridge until the first real group;
                      # any PE idle gap risks losing the HAM 2.4GHz state,
                      # so overshooting (56ns/extra dummy when warm) is cheap

_CACHE = {}


def _build_nc():
    import concourse.bacc as bacc
    import concourse.tile as tile
    import concourse.bass as bass
    from concourse import mybir

    f32 = mybir.dt.float32
    bf16 = mybir.dt.bfloat16
    fp8 = mybir.dt.float8e4
    attn_dt = fp8 if USE_FP8 else bf16
    out_dt = fp8 if OUT_FP8 else bf16
    OP = mybir.AluOpType
    AF = mybir.ActivationFunctionType

    nc = bacc.Bacc(
        "TRN2",
        target_bir_lowering=False,
        debug=False,
        num_devices=N_CORES,
        enable_partition_id=False,
    )

    attn_in = nc.dram_tensor(
        "attn_in", [N_GROUPS * P, HW], attn_dt, kind="ExternalInput"
    ).ap()
    grid_in = nc.dram_tensor(
        "grid_in", [P, GRID_COLS], bf16, kind="ExternalInput"
    ).ap()
    # raw 80-col histograms per group; host extracts the block diagonal
    cd_out = nc.dram_tensor(
        "cd_out", [P, N_GROUPS * NC80], out_dt, kind="ExternalOutput"
    ).ap()

    with tile.TileContext(nc) as tc:
        with ExitStack() as ctx:
            singles = ctx.enter_context(tc.tile_pool(name="singles", bufs=1))
            psum = ctx.enter_context(
                tc.tile_pool(name="psum", bufs=4, space="PSUM")
            )
            psum_d = ctx.enter_context(
                tc.tile_pool(name="psum_d", bufs=1, space="PSUM")
            )

            # ---- PE pre-warm: memset a scratch tile, then dummy matmuls
            # keep the PE busy from ~7.6us so the HAM clock gate is at 8/8
            # (2.4 GHz) when the first real group lands.
            dummy_w = singles.tile([P, P], attn_dt, name="dummy_w")
            nc.gpsimd.memset(dummy_w, 0)
            ps_d = psum_d.tile([P, 512], f32, name="ps_d")
            for _ in range(N_WARM):
                nc.tensor.matmul(ps_d[:, 0:P], dummy_w, dummy_w,
                                 start=True, stop=True)

            # ---- grid: first DMA on each HWDGE ring -> lands ~9.4us
            gridT = singles.tile([P, GRID_COLS], bf16)
            half = GRID_COLS // 2
            nc.sync.dma_start(out=gridT[:, 0:half], in_=grid_in[:, 0:half])
            nc.scalar.dma_start(
                out=gridT[:, half:GRID_COLS], in_=grid_in[:, half:GRID_COLS]
            )

            # ---- attn: column-split across both HWDGE rings so groups
            # complete serially at aggregate bandwidth.
            HALF_HW = HW // 2  # chunks 0-15 on sync, 16-31 on scalar
            attn_sb = []
            for g in range(N_GROUPS - 1):
                t = singles.tile([P, HW], attn_dt, name=f"attn{g}")
                attn_sb.append(t)
                rows = slice(g * P, (g + 1) * P)
                nc.sync.dma_start(
                    out=t[:, 0:HALF_HW], in_=attn_in[rows, 0:HALF_HW]
                )
                nc.scalar.dma_start(
                    out=t[:, HALF_HW:HW], in_=attn_in[rows, HALF_HW:HW]
                )
            # last group in three pieces with SEPARATE tiles: Tile's
            # DMA->matmul dependency is per-tile, so with one shared tile
            # all of g7's matmuls would gate on the LAST piece's receipt.
            # With own tiles only 4 matmuls trail the stream's final 64KB.
            g7r = slice((N_GROUPS - 1) * P, N_GROUPS * P)
            K1, K2 = 16, 28
            t7b = singles.tile([P, (K2 - K1) * CHUNK], attn_dt, name="a7b")
            t7a = singles.tile([P, K1 * CHUNK], attn_dt, name="a7a")
            t7c = singles.tile([P, (N_CHUNKS - K2) * CHUNK], attn_dt,
                               name="a7c")
            nc.scalar.dma_start(
                out=t7b, in_=attn_in[g7r, K1 * CHUNK : K2 * CHUNK]
            )
            nc.sync.dma_start(out=t7a, in_=attn_in[g7r, 0 : K1 * CHUNK])
            nc.sync.dma_start(out=t7c, in_=attn_in[g7r, K2 * CHUNK : HW])

            def stat_slice(g, k):
                if g < N_GROUPS - 1:
                    return attn_sb[g][:, k * CHUNK : (k + 1) * CHUNK]
                t, k0 = (
                    (t7a, 0) if k < K1
                    else (t7b, K1) if k < K2
                    else (t7c, K2)
                )
                return t[:, (k - k0) * CHUNK : (k - k0 + 1) * CHUNK]

            cdq_all = singles.tile([P, N_GROUPS * NC80], out_dt)

            # ---- one-hot masks, color-major per group-pair ----
            # mask_pair[p, c*512 + g2*256 + k*8 + b] = (grid[p,(g,k,b)] == c)
            # Ten contiguous-slice is_equal tensor_scalar ops per pair:
            # single-src + bf16 + contiguous -> DVE 4x perf mode.
            mask_pairs = []
            for pr in range(N_GROUPS // 2):
                mq = singles.tile(
                    [P, NUM_COLORS * PAIR_COLS], bf16, name=f"maskp{pr}"
                )
                src = gridT[:, pr * PAIR_COLS : (pr + 1) * PAIR_COLS]
                for c in range(NUM_COLORS):
                    nc.vector.tensor_scalar(
                        out=mq[:, c * PAIR_COLS : (c + 1) * PAIR_COLS],
                        in0=src,
                        scalar1=float(c),
                        scalar2=None,
                        op0=OP.is_equal,
                    )
                mask_pairs.append(mq)

            def moving_ap(g, k):
                # 80 cols (c,b') for chunk k of group g: strided read into
                # the c-major pair mask; col order c*8+b matches v5 layout
                mq = mask_pairs[g // 2][:, :]
                return bass.AP(
                    tensor=mq.tensor,
                    offset=mq.offset + (g % 2) * (PAIR_COLS // 2)
                    + k * B_PER_GROUP,
                    ap=[mq.ap[0], [PAIR_COLS, NUM_COLORS], [1, B_PER_GROUP]],
                )

            for g in range(N_GROUPS):
                ps = psum.tile([P, 512], f32, name="ps", tag="ps")
                ps_c = ps[:, 0:NC80]
                if g < N_GROUPS - 1:
                    chunk_order = list(range(N_CHUNKS))
                else:
                    # match the 3-piece DMA completion order: scalar's
                    # 16-27 land first, then sync's 0-15, then 28-31
                    chunk_order = (
                        list(range(16, 28))
                        + list(range(0, 16))
                        + list(range(28, 32))
                    )
                for i, k in enumerate(chunk_order):
                    nc.tensor.matmul(
                        ps_c,
                        stat_slice(g, k),
                        moving_ap(g, k),
                        start=(i == 0),
                        stop=(i == N_CHUNKS - 1),
                    )

                # the only post-op: PSUM -> SBUF (fp8) on the ACT engine.
                # scale 1/16 keeps cd well inside fp8e4's range; the host
                # entropy is scale-invariant so no decode step is needed.
                nc.scalar.activation(
                    cdq_all[:, g * NC80 : (g + 1) * NC80], ps_c, AF.Copy,
                    scale=(1.0 / 16.0 if OUT_FP8 else 1.0),
                )
                if g == N_GROUPS - 2:
                    # ship groups 0-6 while group 7 computes; only the last
                    # 80 columns remain on the critical tail
                    cut = (N_GROUPS - 1) * NC80
                    nc.sync.dma_start(
                        out=cd_out[:, 0:cut], in_=cdq_all[:, 0:cut]
                    )

            # last piece goes out on the ACT ring right behind its copy
            cut = (N_GROUPS - 1) * NC80
            nc.scalar.dma_start(
                out=cd_out[:, cut : N_GROUPS * NC80],
                in_=cdq_all[:, cut : N_GROUPS * NC80],
            )

    nc.compile()
    return nc


def _get_nc():
    if "nc" not in _CACHE:
        _CACHE["nc"] = _build_nc()
    return _CACHE["nc"]


def _make_in_maps(attn_weights, grids):
    import ml_dtypes

    attn_np_dt = ml_dtypes.float8_e4m3 if USE_FP8 else ml_dtypes.bfloat16
    attn = np.asarray(attn_weights, dtype=np.float32)   # [512,16,64,64]
    grid = np.asarray(grids)                            # [512,64,64]
    in_maps = []
    for c in range(N_CORES):
        lo = c * B_PER_CORE
        # (g,b,s,k,p) -> (g,p,k,b,s): row g*128+p, col k*128 + b*16 + s
        a = attn[lo : lo + B_PER_CORE].reshape(
            N_GROUPS, B_PER_GROUP, S, N_CHUNKS, CHUNK
        )
        a = np.ascontiguousarray(a.transpose(0, 4, 3, 1, 2)).reshape(
            N_GROUPS * CHUNK, HW
        )
        # (g,b,k,p) -> (p,g,k,b): col g*256 + k*8 + b
        g_ = grid[lo : lo + B_PER_CORE].reshape(
            N_GROUPS, B_PER_GROUP, N_CHUNKS, CHUNK
        )
        g_ = np.ascontiguousarray(g_.transpose(3, 0, 2, 1)).reshape(
            CHUNK, GRID_COLS
        )
        in_maps.append(
            {
                "attn_in": a.astype(attn_np_dt),
                "grid_in": g_.astype(np.float32).astype(ml_dtypes.bfloat16),
            }
        )
    return in_maps


def kernel(attn_weights: np.ndarray, grids: np.ndarray) -> np.ndarray:
    from concourse.bass_utils import run_bass_kernel_spmd

    nc = _get_nc()
    in_maps = _make_in_maps(attn_weights, grids)
    res = run_bass_kernel_spmd(nc, in_maps, core_ids=list(range(N_CORES)))

    # Host finale: pick the block diagonal (row p=(b,s) owns batch p//16
    # of its group), then entropy + mean in f64.
    b_idx = (np.arange(P) // S)[:, None, None, None]     # [128,1,1,1]
    total = 0.0
    for c in range(N_CORES):
        r = res.results[c]["cd_out"].astype(np.float64)  # [128, 640]
        r = r.reshape(P, N_GROUPS, NUM_COLORS, B_PER_GROUP)
        cd = np.take_along_axis(r, b_idx, axis=3)[..., 0]  # [128, 8, 10]
        s_ = cd.sum(-1) + EPS
        p_ = cd / s_[..., None]
        ent = -(p_ * np.log(p_ + EPS)).sum(-1)
        total += float(ent.sum())
    return np.float32(total / (B * S))
